# revision 52
# baseline (speedup 1.0000x reference)
"""MoE runtime-experts kernel for 8 Trainium2 NeuronCores.

Problem: y[t] = gelu(x[t] @ W1[e] + b1[e]) @ W2[e] + b2[e], e = indices[t].
T=8192 tokens, D=1024, H=4096, E=8 experts.

Strategy: expert-parallel. Host routes tokens by expert (argsort), core e
gets expert e's weights plus its tokens (transposed, zero-padded to a
common Tp so all 8 cores run one SPMD program). On device each core runs a
dense 2-layer MLP in fp8e4m3 with DoubleRow matmuls (256-deep contraction
per pass, ~1.9x bf16 PE throughput) and fp32 PSUM accumulation:

  layer 1: hT[h, t] = gelu(sum_d W1[d, h] * xT[d, t] + b1[h])
           (lhsT = W1 k-pair [128d, 2, 128h], rhs = xT [128d, 2, 384t])
  layer 2: yT[d, t] = sum_h W2[h, d] * hT[h, t] + b2[d]
           (lhsT = W2 h-pair [128h, 2, 128d], rhs = hT [128h, 2, 384t])

fp8 accuracy (rel err 0.0022 vs 2e-2 gate, same order as bf16) hinges on
sum-preserving error-diffusion quantization of x on the host: plain RTN
x-error couples to W1's all-positive column means into a token-correlated
error that W2's positive mean amplifies ~10x past the gate (measured
0.0225); preserving each token's feature sum kills it at zero device cost
(see _ed_quant_rows). h is quantized to fp8 by the GELU ACT on device;
y stores as bf16 (halves output DMA).

Both layers keep the token axis in the free dimension, so no on-device
transpose is needed; every core computes exactly max(counts) token
columns, split into even-sized chunks of <=384 (one fp32 PSUM bank each).
Engine layout: Scalar queue runs only GELU (plus tiny startup DMAs), the
sync ring streams w1, the gpsimd ring carries x chunks + w2, output
bias-adds run on the DVE, and stores round-robin all three rings. A boot
DMA packs w1[0]/chunk-0 k-pair 0 so one semaphore gates the first matmul.
Host un-permutes yT shards into the full [T, 1, D] fp32 output.

Measured on 8 axon trn2 cores: ~138us (PE matmul busy ~121us at the fp8
DoubleRow hardware floor of 1 column/cycle; the rest is runtime
preamble/HAM/drain overhead). Warmup tricks: dependency-free dummy
matmuls on a zeroed tile hide the first DMA-completion semaphore wake
(~2.7us), and the first 6 h-tiles run chunk 0 only so chunks 1-2 have
extra time to land in the bandwidth-bound early phase (~1us). bf16
baseline of the same structure: 258us.

KERNEL_MODE selects compute dtype: "fp8" (default: both layers fp8e4m3 +
DoubleRow), "fp8l1" (layer 1 fp8, layer 2 bf16), "bf16".
"""

import math
import os

import numpy as np
import ml_dtypes

T, D, H, E = 8192, 1024, 4096, 8
N_CORES = 8
KB_D = D // 128  # 8  k-tiles of the D contraction
HB = H // 128  # 32 h-tiles
DB = D // 128  # 8  d-tiles
BF16 = ml_dtypes.bfloat16
CS = 384  # token chunk (matmul moving-operand free dim)
SUP = 4 * CS  # tokens resident per pass (SBUF limit)
MM_N = 512  # PSUM bank free size (fp32)

MODE = os.environ.get("KERNEL_MODE", "fp8")

_program_cache: dict[tuple, object] = {}
last_results = None  # BassKernelResults of the most recent kernel() call


def _ed_quant_rows(a: np.ndarray, fp8_np) -> np.ndarray:
    """Error-diffusion quantize each row of [N, D] to fp8, carrying the
    rounding error along D so the row sum is preserved. Plain
    round-to-nearest x-quantization error couples to W1's all-positive
    column means into a token-correlated error that layer 2's all-positive
    W2 amplifies ~10x past the accuracy gate; sum-preserving quantization
    kills that term at zero device cost (measured: rel 0.0225 -> 0.0014)."""
    a = a.astype(np.float32)
    out = np.empty(a.shape, np.float32)
    carry = np.zeros(a.shape[0], np.float32)
    for d in range(a.shape[1]):
        v = a[:, d] + carry
        q = v.astype(fp8_np).astype(np.float32)
        out[:, d] = q
        carry = v - q
    return out


def _chunk_sizes(Tp: int):
    """Balanced split of Tp token columns into chunks of at most CS.
    Sizes are kept even: odd moving-dims measure ~2% slower per column
    on the PE (alignment penalty). A maximal first chunk (512) was
    tried and measured WORSE (+1.7us): the warmup phase is
    HBM-bandwidth-bound, so a heavier first group delays the pipeline."""
    nch = max(1, math.ceil(Tp / CS))
    base, rem = divmod(Tp, nch)
    sizes = [base + (1 if i < rem else 0) for i in range(nch)]
    for i in range(nch - 1):
        if sizes[i] % 2:
            sizes[i] += 1
            sizes[i + 1] -= 1
    return sizes


def _build_program(Tp: int, mode: str):
    import concourse.tile as tile
    from concourse import bacc, mybir

    sizes = _chunk_sizes(Tp)
    nch = len(sizes)
    offs = [sum(sizes[:i]) for i in range(nch)]  # global token offsets

    f32 = mybir.dt.float32
    bf16 = mybir.dt.bfloat16
    fp8 = mybir.dt.float8e4
    l1_dt = fp8 if mode in ("fp8", "fp8l1") else bf16
    l2_dt = fp8 if mode == "fp8" else bf16
    l1_dr = l1_dt == fp8
    l2_dr = l2_dt == fp8
    dr = mybir.MatmulPerfMode.DoubleRow
    gelu = mybir.ActivationFunctionType.Gelu
    ident = mybir.ActivationFunctionType.Identity

    nc = bacc.Bacc(
        "TRN2", target_bir_lowering=False, debug=False, num_devices=N_CORES
    )

    # xq[c] is the SBUF image of token chunk c: [128, KB_D*CS], row-major
    # (kb, t) per partition, so the DMA is fully contiguous
    xq = nc.dram_tensor(
        "xq", [nch, 128, KB_D * CS], l1_dt, kind="ExternalInput"
    ).ap()
    # boot holds w1[0] k-pair 0 (cols 0:128) + chunk-0 k-pair 0 (cols
    # 128:128+CS): ONE dma + ONE semaphore gates the very first matmul
    boot = nc.dram_tensor(
        "boot", [128, 2, 128 + CS], l1_dt, kind="ExternalInput"
    ).ap()
    # w1[hb] packs TWO h-tiles contiguously per partition (2KB lines):
    # halves the DMA count, descriptor count and completion semaphores
    # on the sync ring vs per-tile loads
    w1 = nc.dram_tensor(
        "w1", [HB // 2, 128, 2 * KB_D * 128], l1_dt, kind="ExternalInput"
    ).ap()
    # w2[db] likewise packs two d-tiles per DMA on the gpsimd ring
    w2 = nc.dram_tensor(
        "w2", [DB // 2, 128, 2 * HB * 128], l2_dt, kind="ExternalInput"
    ).ap()
    b1 = nc.dram_tensor("b1", [128, HB], f32, kind="ExternalInput").ap()
    b2 = nc.dram_tensor("b2", [128, DB], f32, kind="ExternalInput").ap()
    # bf16 output halves the store traffic; the add runs fp32 on DVE and
    # only the final store rounds (costs ~1e-3 rel err, gate is 2e-2)
    yT = nc.dram_tensor("yT", [D, Tp], bf16, kind="ExternalOutput").ap()

    def mm_group(ps, tsz, nk, lhs_of, rhs_of, use_dr):
        """Accumulate nk k-tiles into psum ps[:, :tsz]; DoubleRow fuses
        pairs of k-tiles per matmul via 3D APs."""
        if use_dr:
            for j in range(0, nk, 2):
                nc.tensor.matmul(
                    ps[:, :tsz],
                    lhs_of(j, 2),
                    rhs_of(j, 2),
                    start=(j == 0),
                    stop=(j == nk - 2),
                    perf_mode=dr,
                )
        else:
            for j in range(nk):
                nc.tensor.matmul(
                    ps[:, :tsz],
                    lhs_of(j, 1),
                    rhs_of(j, 1),
                    start=(j == 0),
                    stop=(j == nk - 1),
                )

    with tile.TileContext(nc) as tc:
        with (
            tc.tile_pool(name="const", bufs=1) as const_pool,
            tc.tile_pool(name="acts", bufs=1) as acts_pool,
            tc.tile_pool(name="xtp", bufs=3) as xt_pool,
            tc.tile_pool(name="w1p", bufs=6) as w1_pool,
            tc.tile_pool(name="w2p", bufs=2) as w2_pool,
            tc.tile_pool(name="outp", bufs=4) as out_pool,
            tc.tile_pool(name="psum", bufs=8, space="PSUM") as psum_pool,
        ):
            b1_sb = const_pool.tile([128, HB], f32)
            b2_sb = const_pool.tile([128, DB], f32)

            for sup0 in range(0, nch, SUP // CS):

                cix = list(range(sup0, min(sup0 + SUP // CS, nch)))
                loffs = [offs[c] - offs[cix[0]] for c in cix]  # ht-local
                sup_len = sum(sizes[c] for c in cix)
                ht_sb = acts_pool.tile([128, HB, sup_len], l2_dt, tag="ht")

                # token chunks: chunk 0 on the scalar ring (gates the first
                # matmul; Scalar is idle until GELUs begin), the rest on the
                # gpsimd ring in parallel; the sync ring carries the w1
                # stream so Scalar stays dedicated to GELU afterwards
                # the very first matmul group reads w1[0] k-pair 0 and
                # chunk-0 k-pair 0 out of the boot tile: one DMA, one
                # completion semaphore, first on the gpsimd (SWDGE) ring
                # whose descriptor writes issue earliest and cheapest
                boot_sb = None
                if sup0 == 0 and l1_dr:
                    boot_sb = const_pool.tile([128, 2, 128 + CS], l1_dt)
                    # memset on the gpsimd queue: it exits the preamble
                    # earliest, so the dummy-matmul chain starts ~0.5us
                    # sooner than with the vector queue
                    garb = const_pool.tile([128, 2, 512], l1_dt)
                    nc.gpsimd.memset(garb[:], 0)
                    nc.gpsimd.dma_start(boot_sb[:], boot[:])
                    # dependency-free warmup matmuls on (uninitialized)
                    # SBUF: the PE starts immediately and stays busy while
                    # the boot DMA lands, so the first real matmul's wait
                    # is already satisfied when evaluated — hiding the
                    # ~3us DMA-completion semaphore wake latency. Results
                    # land in a psum tile no real group reads; real
                    # groups reset their banks via start=True.
                    warm_ps = psum_pool.tile([128, MM_N], f32, tag="ps")
                    for wi in range(7):
                        nc.tensor.matmul(
                            warm_ps[:, :MM_N],
                            garb[:, :, :128],
                            garb[:, :, :],
                            start=(wi == 0),
                            stop=(wi == 6),
                            perf_mode=dr,
                        )

                xts = []
                for ci, c in enumerate(cix):
                    xt_c = xt_pool.tile(
                        [128, KB_D, CS], l1_dt, tag=f"xt{ci}", bufs=1
                    )
                    src = xq[c].rearrange("p (k m) -> p k m", k=KB_D)
                    if ci == 0:
                        # the first matmul group reads its k-pair 0 from
                        # the boot copy; these normal loads feed h>=1
                        nc.gpsimd.dma_start(xt_c[:, :2], src[:, :2])
                        nc.gpsimd.dma_start(xt_c[:, 2:], src[:, 2:])
                    elif ci == 1:
                        # halves unblock h=0's chunk groups sooner;
                        # chunk 1 rides the scalar ring (idle til GELU)
                        nc.scalar.dma_start(xt_c[:, :4], src[:, :4])
                        nc.scalar.dma_start(xt_c[:, 4:], src[:, 4:])
                    else:
                        nc.gpsimd.dma_start(xt_c[:, :4], src[:, :4])
                        nc.gpsimd.dma_start(xt_c[:, 4:], src[:, 4:])
                    xts.append(xt_c)
                if sup0 == 0:
                    nc.scalar.dma_start(b1_sb[:], b1[:])
                    nc.scalar.dma_start(b2_sb[:], b2[:])

                # ---- layer 1: hT[h, c] ----
                # cell order: the first PA h-tiles run chunk 0 only, with
                # their chunk-1/2 cells deferred — the early phase is
                # HBM-bandwidth-bound, and this gives chunks 1-2 an extra
                # ~3us to land before the PE needs them
                PA = 6
                if len(cix) == 3 and sup0 == 0 and HB >= PA:
                    # interleave the tail of phase A with the first
                    # deferred cells: spreads the tripled w1-block demand
                    # (the 1.4us bubble was w1 starvation, not x)
                    cells = (
                        [(h, 0) for h in range(4)]
                        + [(0, 1), (4, 0), (0, 2), (5, 0)]
                        + [
                            (h, ci)
                            for h in range(1, PA)
                            for ci in range(1, len(cix))
                        ]
                        + [
                            (h, ci)
                            for h in range(PA, HB)
                            for ci in range(len(cix))
                        ]
                    )
                elif len(cix) > 1 and sup0 == 0:
                    cells = (
                        [(h, 0) for h in range(PA)]
                        + [
                            (h, ci)
                            for h in range(PA)
                            for ci in range(1, len(cix))
                        ]
                        + [
                            (h, ci)
                            for h in range(PA, HB)
                            for ci in range(len(cix))
                        ]
                    )
                else:
                    cells = [
                        (h, ci)
                        for h in range(HB)
                        for ci in range(len(cix))
                    ]
                w1blocks = {}
                for h, ci in cells:
                    # w1 stream rides the sync ring so the Scalar queue
                    # stays dedicated to GELU (it was 75% busy with DMA
                    # descriptors + semaphores stealing from ACT)
                    blk = h // 2
                    if blk not in w1blocks:
                        w1t2 = w1_pool.tile(
                            [128, 2, KB_D, 128], l1_dt, tag="w1t"
                        )
                        nc.sync.dma_start(
                            w1t2[:],
                            w1[blk].rearrange(
                                "p (b k m) -> p b k m", b=2, k=KB_D
                            ),
                        )
                        w1blocks[blk] = w1t2
                    w1t = w1blocks[blk][:, h % 2]
                    for ci in [ci]:
                        c = cix[ci]
                        xt_c = xts[ci]
                        tsz = sizes[c]
                        lo = loffs[ci]
                        ps = psum_pool.tile([128, MM_N], f32, tag="ps")
                        if boot_sb is not None and h == 0 and ci == 0:
                            bs = boot_sb

                            def lhs_of(j, w, _w1t=w1t, _bs=bs):
                                if j == 0 and w == 2:
                                    return _bs[:, :, :128]
                                return (
                                    _w1t[:, j : j + w, :]
                                    if w == 2
                                    else _w1t[:, j, :]
                                )

                            def rhs_of(j, w, _xt=xt_c, _bs=bs, _t=tsz):
                                if j == 0 and w == 2:
                                    return _bs[:, :, 128 : 128 + _t]
                                return (
                                    _xt[:, j : j + w, :_t]
                                    if w == 2
                                    else _xt[:, j, :_t]
                                )

                            mm_group(ps, tsz, KB_D, lhs_of, rhs_of, l1_dr)
                        else:
                            mm_group(
                                ps,
                                tsz,
                                KB_D,
                                lambda j, w: w1t[:, j : j + w, :]
                                if w == 2
                                else w1t[:, j, :],
                                lambda j, w: xt_c[:, j : j + w, :tsz]
                                if w == 2
                                else xt_c[:, j, :tsz],
                                l1_dr,
                            )
                        nc.scalar.activation(
                            ht_sb[:, h, lo : lo + tsz],
                            ps[:, :tsz],
                            gelu,
                            bias=b1_sb[:, h : h + 1],
                        )

                # ---- layer 2: yT[d, c] ----
                w2t2 = None
                for d in range(DB):
                    # w2 on the gpsimd (SWDGE) ring: parallel to the w1
                    # stream on the sync ring, so d=0 prefetches early
                    if d % 2 == 0:
                        w2t2 = w2_pool.tile(
                            [128, 2, HB, 128], l2_dt, tag="w2t"
                        )
                        nc.gpsimd.dma_start(
                            w2t2[:],
                            w2[d // 2].rearrange(
                                "p (b k m) -> p b k m", b=2, k=HB
                            ),
                        )
                    w2t = w2t2[:, d % 2]
                    rings = [nc.sync, nc.scalar, nc.gpsimd]
                    # d < DB-1: one DVE add per chunk into a full-width
                    # tile, then ONE store DMA per d-tile (fewer
                    # descriptors + completion semaphores). The final
                    # d-tile keeps fine-grained pieces so the exposed
                    # tail after the last matmul stays short.
                    lo0 = offs[cix[0]]
                    hi0 = offs[cix[-1]] + sizes[cix[-1]]
                    ot_d = None
                    if d < DB - 1:
                        ot_d = out_pool.tile(
                            [128, hi0 - lo0], bf16, tag="otd"
                        )
                    for ci, c in enumerate(cix):
                        tsz = sizes[c]
                        lo = loffs[ci]
                        go = offs[c]
                        # the very last matmul group is split column-wise
                        # so its first half's add+store overlap the second
                        # half's matmuls, shrinking the exposed tail
                        last = d == DB - 1 and c == cix[-1]
                        col_ranges = (
                            [(0, tsz // 2), (tsz // 2, tsz)]
                            if last
                            else [(0, tsz)]
                        )
                        for r0, r1 in col_ranges:
                            rn = r1 - r0
                            ps = psum_pool.tile([128, MM_N], f32, tag="ps")
                            mm_group(
                                ps,
                                rn,
                                HB,
                                lambda j, w: w2t[:, j : j + w, :]
                                if w == 2
                                else w2t[:, j, :],
                                lambda j, w: ht_sb[
                                    :, j : j + w, lo + r0 : lo + r1
                                ]
                                if w == 2
                                else ht_sb[:, j, lo + r0 : lo + r1],
                                l2_dr,
                            )
                            if d < DB - 1:
                                nc.vector.tensor_scalar_add(
                                    ot_d[:, go - lo0 : go - lo0 + tsz],
                                    ps[:, :tsz],
                                    b2_sb[:, d : d + 1],
                                )
                                continue
                            ot = out_pool.tile([128, MM_N], bf16, tag="ot")
                            # final d-tile: split each range's add+store
                            # into pieces on alternating rings so no ring
                            # backlog extends the tail
                            if rn > 160:
                                half = rn // 2
                                pieces = [(0, half), (half, rn - half)]
                            else:
                                pieces = [(0, rn)]
                            for pi, (p0, psz) in enumerate(pieces):
                                # bias add on DVE, not Scalar ACT: keeps
                                # the Scalar queue free and overlaps the
                                # layer-2 tail
                                nc.vector.tensor_scalar_add(
                                    ot[:, p0 : p0 + psz],
                                    ps[:, p0 : p0 + psz],
                                    b2_sb[:, d : d + 1],
                                )
                                st_eng = rings[
                                    (d * len(cix) + ci + pi + r0) % 3
                                ]
                                st_eng.dma_start(
                                    yT[
                                        d * 128 : (d + 1) * 128,
                                        go + r0 + p0 : go + r0 + p0 + psz,
                                    ],
                                    ot[:, p0 : p0 + psz],
                                )
                    if d < DB - 1:
                        rings[d % 3].dma_start(
                            yT[d * 128 : (d + 1) * 128, lo0:hi0],
                            ot_d[:, : hi0 - lo0],
                        )

    nc.compile()
    return nc


def kernel(x, indices_s, weight1, weight2, bias1, bias2):
    from concourse import mybir
    from concourse.bass_utils import run_bass_kernel_spmd

    x = np.asarray(x, dtype=np.float32)
    if MODE in ("fp8", "fp8l1"):
        x = _ed_quant_rows(x, mybir.dt.np(mybir.dt.float8e4))
    idx = np.asarray(indices_s).astype(np.int64).ravel()
    w1_full = np.asarray(weight1, dtype=np.float32)
    w2_full = np.asarray(weight2, dtype=np.float32)
    b1_full = np.asarray(bias1, dtype=np.float32)
    b2_full = np.asarray(bias2, dtype=np.float32)

    order = np.argsort(idx, kind="stable")
    counts = np.bincount(idx, minlength=E)
    starts = np.concatenate([[0], np.cumsum(counts)])
    # tokens live in the free dim everywhere, so no alignment is needed:
    # every core computes exactly max(counts) token columns
    Tp = max(128, int(counts.max()))
    sizes = _chunk_sizes(Tp)
    nch = len(sizes)
    offs = np.concatenate([[0], np.cumsum(sizes)])

    mode = MODE
    key = (Tp, mode)
    nc = _program_cache.get(key)
    if nc is None:
        nc = _build_program(Tp, mode)
        _program_cache[key] = nc

    fp8_np = mybir.dt.np(mybir.dt.float8e4)
    l1_np = fp8_np if mode in ("fp8", "fp8l1") else BF16
    l2_np = fp8_np if mode == "fp8" else BF16

    in_maps = []
    for e in range(E):
        toks = order[starts[e] : starts[e + 1]]
        # slot-aligned image: chunk c's tokens at columns [c*CS, c*CS+sizes[c])
        xTs = np.zeros((D, nch * CS), dtype=np.float32)
        for c in range(nch):
            lo, hi = offs[c], min(offs[c + 1], counts[e])
            if hi > lo:
                xTs[:, c * CS : c * CS + (hi - lo)] = x[toks[lo:hi]].T
        # [D, nch*CS] -> [nch, 128, KB_D*CS] chunk-major SBUF image
        xq = (
            np.ascontiguousarray(
                xTs.reshape(KB_D, 128, nch, CS).transpose(2, 1, 0, 3)
            )
            .reshape(nch, 128, KB_D * CS)
            .astype(l1_np)
        )
        w1r = (
            np.ascontiguousarray(
                w1_full[e].reshape(KB_D, 128, HB, 128).transpose(2, 1, 0, 3)
            )
            .reshape(HB, 128, KB_D * 128)
            .astype(l1_np)
        )
        w2r = (
            np.ascontiguousarray(
                w2_full[e].reshape(HB, 128, DB, 128).transpose(2, 1, 0, 3)
            )
            .reshape(DB, 128, HB * 128)
            .astype(l2_np)
        )
        # pack pairs of tiles contiguously per partition: one DMA (and
        # one completion semaphore) covers two h-/d-tiles
        w1p = np.ascontiguousarray(
            w1r.reshape(HB // 2, 2, 128, KB_D * 128)
            .transpose(0, 2, 1, 3)
            .reshape(HB // 2, 128, 2 * KB_D * 128)
        )
        w2p = np.ascontiguousarray(
            w2r.reshape(DB // 2, 2, 128, HB * 128)
            .transpose(0, 2, 1, 3)
            .reshape(DB // 2, 128, 2 * HB * 128)
        )
        b1d = np.ascontiguousarray(b1_full[e].reshape(HB, 128).T)
        b2d = np.ascontiguousarray(b2_full[e].reshape(DB, 128).T)
        # boot: w1[0] k-pair 0 next to chunk-0 k-pair 0 so one DMA (and
        # one completion semaphore) gates the very first matmul
        bootd = np.concatenate(
            [
                w1r[0].reshape(128, KB_D, 128)[:, :2, :],
                xq[0].reshape(128, KB_D, CS)[:, :2, :],
            ],
            axis=2,
        )
        bootd = np.ascontiguousarray(bootd)
        in_maps.append(
            {
                "xq": xq,
                "w1": w1p,
                "w2": w2p,
                "b1": b1d,
                "b2": b2d,
                "boot": bootd,
            }
        )

    res = run_bass_kernel_spmd(
        nc,
        in_maps,
        list(range(N_CORES)),
        trace=os.environ.get("BASS_TRACE") == "1",
    )
    global last_results
    last_results = res

    out = np.empty((T, D), dtype=np.float32)
    for e in range(E):
        toks = order[starts[e] : starts[e + 1]]
        out[toks] = res.results[e]["yT"][:, : counts[e]].T.astype(np.float32)
    if res.exec_time_ns is not None:
        print(f"HW exec time: {res.exec_time_ns} ns")
    return out[:, None, :]



# revision 53
# speedup vs baseline: 1.1853x; 1.1853x over previous
"""MoE runtime-experts kernel for 8 Trainium2 NeuronCores.

Problem: y[t] = gelu(x[t] @ W1[e] + b1[e]) @ W2[e] + b2[e], e = indices[t].
T=8192 tokens, D=1024, H=4096, E=8 experts.

Strategy: expert-parallel. Host routes tokens by expert (argsort), core e
gets expert e's weights plus its tokens (transposed, zero-padded to a
common Tp so all 8 cores run one SPMD program). On device each core runs a
dense 2-layer MLP in fp8e4m3 with DoubleRow matmuls (256-deep contraction
per pass, ~1.9x bf16 PE throughput) and fp32 PSUM accumulation:

  layer 1: hT[h, t] = gelu(sum_d W1[d, h] * xT[d, t] + b1[h])
           (lhsT = W1 k-pair [128d, 2, 128h], rhs = xT [128d, 2, 384t])
  layer 2: yT[d, t] = sum_h W2[h, d] * hT[h, t] + b2[d]
           (lhsT = W2 h-pair [128h, 2, 128d], rhs = hT [128h, 2, 384t])

fp8 accuracy (rel err 0.0022 vs 2e-2 gate, same order as bf16) hinges on
sum-preserving error-diffusion quantization of x on the host: plain RTN
x-error couples to W1's all-positive column means into a token-correlated
error that W2's positive mean amplifies ~10x past the gate (measured
0.0225); preserving each token's feature sum kills it at zero device cost
(see _ed_quant_rows). h is quantized to fp8 by the GELU ACT on device;
y stores as bf16 (halves output DMA).

Both layers keep the token axis in the free dimension, so no on-device
transpose is needed; every core computes exactly max(counts) token
columns, split into even-sized chunks of <=384 (one fp32 PSUM bank each).
Engine layout: Scalar queue runs only GELU (plus tiny startup DMAs), the
sync ring streams w1, the gpsimd ring carries x chunks + w2, output
bias-adds run on the DVE, and stores round-robin all three rings. A boot
DMA packs w1[0]/chunk-0 k-pair 0 so one semaphore gates the first matmul.
Host un-permutes yT shards into the full [T, 1, D] fp32 output.

Measured on 8 axon trn2 cores: ~138us (PE matmul busy ~121us at the fp8
DoubleRow hardware floor of 1 column/cycle; the rest is runtime
preamble/HAM/drain overhead). Warmup tricks: dependency-free dummy
matmuls on a zeroed tile hide the first DMA-completion semaphore wake
(~2.7us), and the first 6 h-tiles run chunk 0 only so chunks 1-2 have
extra time to land in the bandwidth-bound early phase (~1us). bf16
baseline of the same structure: 258us.

KERNEL_MODE selects compute dtype: "fp8" (default: both layers fp8e4m3 +
DoubleRow), "fp8l1" (layer 1 fp8, layer 2 bf16), "bf16".
"""

import math
import os

import numpy as np
import ml_dtypes

T, D, H, E = 8192, 1024, 4096, 8
N_CORES = 8
KB_D = D // 128  # 8  k-tiles of the D contraction
HB = H // 128  # 32 h-tiles
DB = D // 128  # 8  d-tiles
BF16 = ml_dtypes.bfloat16
CS = 384  # token chunk (matmul moving-operand free dim)
SUP = 4 * CS  # tokens resident per pass (SBUF limit)
MM_N = 512  # PSUM bank free size (fp32)

MODE = os.environ.get("KERNEL_MODE", "fp8")

_program_cache: dict[tuple, object] = {}
last_results = None  # BassKernelResults of the most recent kernel() call


def _ed_quant_rows(a: np.ndarray, fp8_np) -> np.ndarray:
    """Error-diffusion quantize each row of [N, D] to fp8, carrying the
    rounding error along D so the row sum is preserved. Plain
    round-to-nearest x-quantization error couples to W1's all-positive
    column means into a token-correlated error that layer 2's all-positive
    W2 amplifies ~10x past the accuracy gate; sum-preserving quantization
    kills that term at zero device cost (measured: rel 0.0225 -> 0.0014)."""
    a = a.astype(np.float32)
    out = np.empty(a.shape, np.float32)
    carry = np.zeros(a.shape[0], np.float32)
    for d in range(a.shape[1]):
        v = a[:, d] + carry
        q = v.astype(fp8_np).astype(np.float32)
        out[:, d] = q
        carry = v - q
    return out


def _chunk_sizes(Tp: int):
    """Balanced split of Tp token columns into chunks of at most CS.
    Sizes are kept even: odd moving-dims measure ~2% slower per column
    on the PE (alignment penalty). A maximal first chunk (512) was
    tried and measured WORSE (+1.7us): the warmup phase is
    HBM-bandwidth-bound, so a heavier first group delays the pipeline."""
    nch = max(1, math.ceil(Tp / CS))
    base, rem = divmod(Tp, nch)
    sizes = [base + (1 if i < rem else 0) for i in range(nch)]
    for i in range(nch - 1):
        if sizes[i] % 2:
            sizes[i] += 1
            sizes[i + 1] -= 1
    return sizes


def _build_program(Tp: int, mode: str):
    import concourse.tile as tile
    from concourse import bacc, mybir

    sizes = _chunk_sizes(Tp)
    nch = len(sizes)
    offs = [sum(sizes[:i]) for i in range(nch)]  # global token offsets

    f32 = mybir.dt.float32
    bf16 = mybir.dt.bfloat16
    fp8 = mybir.dt.float8e4
    l1_dt = fp8 if mode in ("fp8", "fp8l1") else bf16
    l2_dt = fp8 if mode == "fp8" else bf16
    l1_dr = l1_dt == fp8
    l2_dr = l2_dt == fp8
    dr = mybir.MatmulPerfMode.DoubleRow
    gelu = mybir.ActivationFunctionType.Gelu
    ident = mybir.ActivationFunctionType.Identity

    nc = bacc.Bacc(
        "TRN2", target_bir_lowering=False, debug=False, num_devices=N_CORES
    )

    # xq[c] is the SBUF image of token chunk c: [128, KB_D*CS], row-major
    # (kb, t) per partition, so the DMA is fully contiguous
    xq = nc.dram_tensor(
        "xq", [nch, 128, KB_D * CS], l1_dt, kind="ExternalInput"
    ).ap()
    # boot holds w1[0] k-pair 0 (cols 0:128) + chunk-0 k-pair 0 (cols
    # 128:128+CS): ONE dma + ONE semaphore gates the very first matmul
    boot = nc.dram_tensor(
        "boot", [128, 2, 128 + CS], l1_dt, kind="ExternalInput"
    ).ap()
    # w1[hb] packs TWO h-tiles contiguously per partition (2KB lines):
    # halves the DMA count, descriptor count and completion semaphores
    # on the sync ring vs per-tile loads
    w1 = nc.dram_tensor(
        "w1", [HB // 2, 128, 2 * KB_D * 128], l1_dt, kind="ExternalInput"
    ).ap()
    # w2[db] likewise packs two d-tiles per DMA on the gpsimd ring
    w2 = nc.dram_tensor(
        "w2", [DB // 2, 128, 2 * HB * 128], l2_dt, kind="ExternalInput"
    ).ap()
    b1 = nc.dram_tensor("b1", [128, HB], f32, kind="ExternalInput").ap()
    b2 = nc.dram_tensor("b2", [128, DB], f32, kind="ExternalInput").ap()
    # bf16 output halves the store traffic; the add runs fp32 on DVE and
    # only the final store rounds (costs ~1e-3 rel err, gate is 2e-2)
    yT = nc.dram_tensor("yT", [D, Tp], bf16, kind="ExternalOutput").ap()

    def mm_group(ps, tsz, nk, lhs_of, rhs_of, use_dr):
        """Accumulate nk k-tiles into psum ps[:, :tsz]; DoubleRow fuses
        pairs of k-tiles per matmul via 3D APs."""
        if use_dr:
            for j in range(0, nk, 2):
                nc.tensor.matmul(
                    ps[:, :tsz],
                    lhs_of(j, 2),
                    rhs_of(j, 2),
                    start=(j == 0),
                    stop=(j == nk - 2),
                    perf_mode=dr,
                )
        else:
            for j in range(nk):
                nc.tensor.matmul(
                    ps[:, :tsz],
                    lhs_of(j, 1),
                    rhs_of(j, 1),
                    start=(j == 0),
                    stop=(j == nk - 1),
                )

    with tile.TileContext(nc) as tc:
        with (
            tc.tile_pool(name="const", bufs=1) as const_pool,
            tc.tile_pool(name="acts", bufs=1) as acts_pool,
            tc.tile_pool(name="xtp", bufs=3) as xt_pool,
            tc.tile_pool(name="w1p", bufs=6) as w1_pool,
            tc.tile_pool(name="w2p", bufs=2) as w2_pool,
            tc.tile_pool(name="outp", bufs=4) as out_pool,
            tc.tile_pool(name="psum", bufs=8, space="PSUM") as psum_pool,
        ):
            b1_sb = const_pool.tile([128, HB], f32)
            b2_sb = const_pool.tile([128, DB], f32)

            for sup0 in range(0, nch, SUP // CS):

                cix = list(range(sup0, min(sup0 + SUP // CS, nch)))
                loffs = [offs[c] - offs[cix[0]] for c in cix]  # ht-local
                sup_len = sum(sizes[c] for c in cix)
                ht_sb = acts_pool.tile([128, HB, sup_len], l2_dt, tag="ht")

                # token chunks: chunk 0 on the scalar ring (gates the first
                # matmul; Scalar is idle until GELUs begin), the rest on the
                # gpsimd ring in parallel; the sync ring carries the w1
                # stream so Scalar stays dedicated to GELU afterwards
                # the very first matmul group reads w1[0] k-pair 0 and
                # chunk-0 k-pair 0 out of the boot tile: one DMA, one
                # completion semaphore, first on the gpsimd (SWDGE) ring
                # whose descriptor writes issue earliest and cheapest
                boot_sb = None
                if sup0 == 0 and l1_dr:
                    boot_sb = const_pool.tile([128, 2, 128 + CS], l1_dt)
                    # memset must ride the DVE: a gpsimd memset measured
                    # ~26us slower end-to-end (GpSimd is slow at
                    # elementwise AND it stalls the queue that issues the
                    # boot/x/w2 DMA descriptors)
                    garb = const_pool.tile([128, 2, 512], l1_dt)
                    nc.vector.memset(garb[:], 0)
                    nc.gpsimd.dma_start(boot_sb[:], boot[:])
                    # dependency-free warmup matmuls on (uninitialized)
                    # SBUF: the PE starts immediately and stays busy while
                    # the boot DMA lands, so the first real matmul's wait
                    # is already satisfied when evaluated — hiding the
                    # ~3us DMA-completion semaphore wake latency. Results
                    # land in a psum tile no real group reads; real
                    # groups reset their banks via start=True.
                    warm_ps = psum_pool.tile([128, MM_N], f32, tag="ps")
                    for wi in range(7):
                        nc.tensor.matmul(
                            warm_ps[:, :MM_N],
                            garb[:, :, :128],
                            garb[:, :, :],
                            start=(wi == 0),
                            stop=(wi == 6),
                            perf_mode=dr,
                        )

                xts = []
                for ci, c in enumerate(cix):
                    xt_c = xt_pool.tile(
                        [128, KB_D, CS], l1_dt, tag=f"xt{ci}", bufs=1
                    )
                    src = xq[c].rearrange("p (k m) -> p k m", k=KB_D)
                    if ci == 0:
                        # the first matmul group reads its k-pair 0 from
                        # the boot copy; these normal loads feed h>=1
                        nc.gpsimd.dma_start(xt_c[:, :2], src[:, :2])
                        nc.gpsimd.dma_start(xt_c[:, 2:], src[:, 2:])
                    elif ci == 1:
                        # halves unblock h=0's chunk groups sooner;
                        # chunk 1 rides the scalar ring (idle til GELU)
                        nc.scalar.dma_start(xt_c[:, :4], src[:, :4])
                        nc.scalar.dma_start(xt_c[:, 4:], src[:, 4:])
                    else:
                        nc.gpsimd.dma_start(xt_c[:, :4], src[:, :4])
                        nc.gpsimd.dma_start(xt_c[:, 4:], src[:, 4:])
                    xts.append(xt_c)
                if sup0 == 0:
                    nc.scalar.dma_start(b1_sb[:], b1[:])
                    nc.scalar.dma_start(b2_sb[:], b2[:])

                # ---- layer 1: hT[h, c] ----
                # cell order: the first PA h-tiles run chunk 0 only, with
                # their chunk-1/2 cells deferred — the early phase is
                # HBM-bandwidth-bound, and this gives chunks 1-2 an extra
                # ~3us to land before the PE needs them
                PA = 6
                if len(cix) == 3 and sup0 == 0 and HB >= PA:
                    # interleave the tail of phase A with the first
                    # deferred cells: spreads the tripled w1-block demand
                    # (the 1.4us bubble was w1 starvation, not x)
                    cells = (
                        [(h, 0) for h in range(4)]
                        + [(0, 1), (4, 0), (0, 2), (5, 0)]
                        + [
                            (h, ci)
                            for h in range(1, PA)
                            for ci in range(1, len(cix))
                        ]
                        + [
                            (h, ci)
                            for h in range(PA, HB)
                            for ci in range(len(cix))
                        ]
                    )
                elif len(cix) > 1 and sup0 == 0:
                    cells = (
                        [(h, 0) for h in range(PA)]
                        + [
                            (h, ci)
                            for h in range(PA)
                            for ci in range(1, len(cix))
                        ]
                        + [
                            (h, ci)
                            for h in range(PA, HB)
                            for ci in range(len(cix))
                        ]
                    )
                else:
                    cells = [
                        (h, ci)
                        for h in range(HB)
                        for ci in range(len(cix))
                    ]
                w1blocks = {}
                for h, ci in cells:
                    # w1 stream rides the sync ring so the Scalar queue
                    # stays dedicated to GELU (it was 75% busy with DMA
                    # descriptors + semaphores stealing from ACT)
                    blk = h // 2
                    if blk not in w1blocks:
                        w1t2 = w1_pool.tile(
                            [128, 2, KB_D, 128], l1_dt, tag="w1t"
                        )
                        nc.sync.dma_start(
                            w1t2[:],
                            w1[blk].rearrange(
                                "p (b k m) -> p b k m", b=2, k=KB_D
                            ),
                        )
                        w1blocks[blk] = w1t2
                    w1t = w1blocks[blk][:, h % 2]
                    for ci in [ci]:
                        c = cix[ci]
                        xt_c = xts[ci]
                        tsz = sizes[c]
                        lo = loffs[ci]
                        ps = psum_pool.tile([128, MM_N], f32, tag="ps")
                        if boot_sb is not None and h == 0 and ci == 0:
                            bs = boot_sb

                            def lhs_of(j, w, _w1t=w1t, _bs=bs):
                                if j == 0 and w == 2:
                                    return _bs[:, :, :128]
                                return (
                                    _w1t[:, j : j + w, :]
                                    if w == 2
                                    else _w1t[:, j, :]
                                )

                            def rhs_of(j, w, _xt=xt_c, _bs=bs, _t=tsz):
                                if j == 0 and w == 2:
                                    return _bs[:, :, 128 : 128 + _t]
                                return (
                                    _xt[:, j : j + w, :_t]
                                    if w == 2
                                    else _xt[:, j, :_t]
                                )

                            mm_group(ps, tsz, KB_D, lhs_of, rhs_of, l1_dr)
                        else:
                            mm_group(
                                ps,
                                tsz,
                                KB_D,
                                lambda j, w: w1t[:, j : j + w, :]
                                if w == 2
                                else w1t[:, j, :],
                                lambda j, w: xt_c[:, j : j + w, :tsz]
                                if w == 2
                                else xt_c[:, j, :tsz],
                                l1_dr,
                            )
                        nc.scalar.activation(
                            ht_sb[:, h, lo : lo + tsz],
                            ps[:, :tsz],
                            gelu,
                            bias=b1_sb[:, h : h + 1],
                        )

                # ---- layer 2: yT[d, c] ----
                w2t2 = None
                for d in range(DB):
                    # w2 on the gpsimd (SWDGE) ring: parallel to the w1
                    # stream on the sync ring, so d=0 prefetches early
                    if d % 2 == 0:
                        w2t2 = w2_pool.tile(
                            [128, 2, HB, 128], l2_dt, tag="w2t"
                        )
                        nc.gpsimd.dma_start(
                            w2t2[:],
                            w2[d // 2].rearrange(
                                "p (b k m) -> p b k m", b=2, k=HB
                            ),
                        )
                    w2t = w2t2[:, d % 2]
                    rings = [nc.sync, nc.scalar, nc.gpsimd]
                    # d < DB-1: one DVE add per chunk into a full-width
                    # tile, then ONE store DMA per d-tile (fewer
                    # descriptors + completion semaphores). The final
                    # d-tile keeps fine-grained pieces so the exposed
                    # tail after the last matmul stays short.
                    lo0 = offs[cix[0]]
                    hi0 = offs[cix[-1]] + sizes[cix[-1]]
                    ot_d = None
                    if d < DB - 1:
                        ot_d = out_pool.tile(
                            [128, hi0 - lo0], bf16, tag="otd"
                        )
                    for ci, c in enumerate(cix):
                        tsz = sizes[c]
                        lo = loffs[ci]
                        go = offs[c]
                        # the very last matmul group is split column-wise
                        # so its first half's add+store overlap the second
                        # half's matmuls, shrinking the exposed tail
                        last = d == DB - 1 and c == cix[-1]
                        col_ranges = (
                            [(0, tsz // 2), (tsz // 2, tsz)]
                            if last
                            else [(0, tsz)]
                        )
                        for r0, r1 in col_ranges:
                            rn = r1 - r0
                            ps = psum_pool.tile([128, MM_N], f32, tag="ps")
                            mm_group(
                                ps,
                                rn,
                                HB,
                                lambda j, w: w2t[:, j : j + w, :]
                                if w == 2
                                else w2t[:, j, :],
                                lambda j, w: ht_sb[
                                    :, j : j + w, lo + r0 : lo + r1
                                ]
                                if w == 2
                                else ht_sb[:, j, lo + r0 : lo + r1],
                                l2_dr,
                            )
                            if d < DB - 1:
                                nc.vector.tensor_scalar_add(
                                    ot_d[:, go - lo0 : go - lo0 + tsz],
                                    ps[:, :tsz],
                                    b2_sb[:, d : d + 1],
                                )
                                continue
                            ot = out_pool.tile([128, MM_N], bf16, tag="ot")
                            # final d-tile: split each range's add+store
                            # into pieces on alternating rings so no ring
                            # backlog extends the tail
                            if rn > 160:
                                half = rn // 2
                                pieces = [(0, half), (half, rn - half)]
                            else:
                                pieces = [(0, rn)]
                            for pi, (p0, psz) in enumerate(pieces):
                                # bias add on DVE, not Scalar ACT: keeps
                                # the Scalar queue free and overlaps the
                                # layer-2 tail
                                nc.vector.tensor_scalar_add(
                                    ot[:, p0 : p0 + psz],
                                    ps[:, p0 : p0 + psz],
                                    b2_sb[:, d : d + 1],
                                )
                                st_eng = rings[
                                    (d * len(cix) + ci + pi + r0) % 3
                                ]
                                st_eng.dma_start(
                                    yT[
                                        d * 128 : (d + 1) * 128,
                                        go + r0 + p0 : go + r0 + p0 + psz,
                                    ],
                                    ot[:, p0 : p0 + psz],
                                )
                    if d < DB - 1:
                        rings[d % 3].dma_start(
                            yT[d * 128 : (d + 1) * 128, lo0:hi0],
                            ot_d[:, : hi0 - lo0],
                        )

    nc.compile()
    return nc


def kernel(x, indices_s, weight1, weight2, bias1, bias2):
    from concourse import mybir
    from concourse.bass_utils import run_bass_kernel_spmd

    x = np.asarray(x, dtype=np.float32)
    if MODE in ("fp8", "fp8l1"):
        x = _ed_quant_rows(x, mybir.dt.np(mybir.dt.float8e4))
    idx = np.asarray(indices_s).astype(np.int64).ravel()
    w1_full = np.asarray(weight1, dtype=np.float32)
    w2_full = np.asarray(weight2, dtype=np.float32)
    b1_full = np.asarray(bias1, dtype=np.float32)
    b2_full = np.asarray(bias2, dtype=np.float32)

    order = np.argsort(idx, kind="stable")
    counts = np.bincount(idx, minlength=E)
    starts = np.concatenate([[0], np.cumsum(counts)])
    # tokens live in the free dim everywhere, so no alignment is needed:
    # every core computes exactly max(counts) token columns
    Tp = max(128, int(counts.max()))
    sizes = _chunk_sizes(Tp)
    nch = len(sizes)
    offs = np.concatenate([[0], np.cumsum(sizes)])

    mode = MODE
    key = (Tp, mode)
    nc = _program_cache.get(key)
    if nc is None:
        nc = _build_program(Tp, mode)
        _program_cache[key] = nc

    fp8_np = mybir.dt.np(mybir.dt.float8e4)
    l1_np = fp8_np if mode in ("fp8", "fp8l1") else BF16
    l2_np = fp8_np if mode == "fp8" else BF16

    in_maps = []
    for e in range(E):
        toks = order[starts[e] : starts[e + 1]]
        # slot-aligned image: chunk c's tokens at columns [c*CS, c*CS+sizes[c])
        xTs = np.zeros((D, nch * CS), dtype=np.float32)
        for c in range(nch):
            lo, hi = offs[c], min(offs[c + 1], counts[e])
            if hi > lo:
                xTs[:, c * CS : c * CS + (hi - lo)] = x[toks[lo:hi]].T
        # [D, nch*CS] -> [nch, 128, KB_D*CS] chunk-major SBUF image
        xq = (
            np.ascontiguousarray(
                xTs.reshape(KB_D, 128, nch, CS).transpose(2, 1, 0, 3)
            )
            .reshape(nch, 128, KB_D * CS)
            .astype(l1_np)
        )
        w1r = (
            np.ascontiguousarray(
                w1_full[e].reshape(KB_D, 128, HB, 128).transpose(2, 1, 0, 3)
            )
            .reshape(HB, 128, KB_D * 128)
            .astype(l1_np)
        )
        w2r = (
            np.ascontiguousarray(
                w2_full[e].reshape(HB, 128, DB, 128).transpose(2, 1, 0, 3)
            )
            .reshape(DB, 128, HB * 128)
            .astype(l2_np)
        )
        # pack pairs of tiles contiguously per partition: one DMA (and
        # one completion semaphore) covers two h-/d-tiles
        w1p = np.ascontiguousarray(
            w1r.reshape(HB // 2, 2, 128, KB_D * 128)
            .transpose(0, 2, 1, 3)
            .reshape(HB // 2, 128, 2 * KB_D * 128)
        )
        w2p = np.ascontiguousarray(
            w2r.reshape(DB // 2, 2, 128, HB * 128)
            .transpose(0, 2, 1, 3)
            .reshape(DB // 2, 128, 2 * HB * 128)
        )
        b1d = np.ascontiguousarray(b1_full[e].reshape(HB, 128).T)
        b2d = np.ascontiguousarray(b2_full[e].reshape(DB, 128).T)
        # boot: w1[0] k-pair 0 next to chunk-0 k-pair 0 so one DMA (and
        # one completion semaphore) gates the very first matmul
        bootd = np.concatenate(
            [
                w1r[0].reshape(128, KB_D, 128)[:, :2, :],
                xq[0].reshape(128, KB_D, CS)[:, :2, :],
            ],
            axis=2,
        )
        bootd = np.ascontiguousarray(bootd)
        in_maps.append(
            {
                "xq": xq,
                "w1": w1p,
                "w2": w2p,
                "b1": b1d,
                "b2": b2d,
                "boot": bootd,
            }
        )

    res = run_bass_kernel_spmd(
        nc,
        in_maps,
        list(range(N_CORES)),
        trace=os.environ.get("BASS_TRACE") == "1",
    )
    global last_results
    last_results = res

    out = np.empty((T, D), dtype=np.float32)
    for e in range(E):
        toks = order[starts[e] : starts[e + 1]]
        out[toks] = res.results[e]["yT"][:, : counts[e]].T.astype(np.float32)
    if res.exec_time_ns is not None:
        print(f"HW exec time: {res.exec_time_ns} ns")
    return out[:, None, :]



# revision 54
# speedup vs baseline: 1.2025x; 1.0145x over previous
"""MoE runtime-experts kernel for 8 Trainium2 NeuronCores.

Problem: y[t] = gelu(x[t] @ W1[e] + b1[e]) @ W2[e] + b2[e], e = indices[t].
T=8192 tokens, D=1024, H=4096, E=8 experts.

Strategy: expert-parallel. Host routes tokens by expert (argsort), core e
gets expert e's weights plus its tokens (transposed, zero-padded to a
common Tp so all 8 cores run one SPMD program). On device each core runs a
dense 2-layer MLP in fp8e4m3 with DoubleRow matmuls (256-deep contraction
per pass, ~1.9x bf16 PE throughput) and fp32 PSUM accumulation:

  layer 1: hT[h, t] = gelu(sum_d W1[d, h] * xT[d, t] + b1[h])
           (lhsT = W1 k-pair [128d, 2, 128h], rhs = xT [128d, 2, 384t])
  layer 2: yT[d, t] = sum_h W2[h, d] * hT[h, t] + b2[d]
           (lhsT = W2 h-pair [128h, 2, 128d], rhs = hT [128h, 2, 384t])

fp8 accuracy (rel err 0.0022 vs 2e-2 gate, same order as bf16) hinges on
sum-preserving error-diffusion quantization of x on the host: plain RTN
x-error couples to W1's all-positive column means into a token-correlated
error that W2's positive mean amplifies ~10x past the gate (measured
0.0225); preserving each token's feature sum kills it at zero device cost
(see _ed_quant_rows). h is quantized to fp8 by the GELU ACT on device;
y stores as bf16 (halves output DMA).

Both layers keep the token axis in the free dimension, so no on-device
transpose is needed; every core computes exactly max(counts) token
columns, split into even-sized chunks of <=384 (one fp32 PSUM bank each).
Engine layout: Scalar queue runs only GELU (plus tiny startup DMAs), the
sync ring streams w1, the gpsimd ring carries x chunks + w2, output
bias-adds run on the DVE, and stores round-robin all three rings. A boot
DMA packs w1[0]/chunk-0 k-pair 0 so one semaphore gates the first matmul.
Host un-permutes yT shards into the full [T, 1, D] fp32 output.

Measured on 8 axon trn2 cores: ~139-140us (PE matmul busy ~121us, ~83%;
the rest is fixed runtime preamble/drain overhead). bf16 baseline of the
same structure: 258us.

KERNEL_MODE selects compute dtype: "fp8" (default: both layers fp8e4m3 +
DoubleRow), "fp8l1" (layer 1 fp8, layer 2 bf16), "bf16".
"""

import math
import os

import numpy as np
import ml_dtypes

T, D, H, E = 8192, 1024, 4096, 8
N_CORES = 8
KB_D = D // 128  # 8  k-tiles of the D contraction
HB = H // 128  # 32 h-tiles
DB = D // 128  # 8  d-tiles
BF16 = ml_dtypes.bfloat16
CS = 384  # token chunk (matmul moving-operand free dim)
SUP = 4 * CS  # tokens resident per pass (SBUF limit)
MM_N = 512  # PSUM bank free size (fp32)

MODE = os.environ.get("KERNEL_MODE", "fp8")

_program_cache: dict[tuple, object] = {}
last_results = None  # BassKernelResults of the most recent kernel() call


def _ed_quant_rows(a: np.ndarray, fp8_np) -> np.ndarray:
    """Error-diffusion quantize each row of [N, D] to fp8, carrying the
    rounding error along D so the row sum is preserved. Plain
    round-to-nearest x-quantization error couples to W1's all-positive
    column means into a token-correlated error that layer 2's all-positive
    W2 amplifies ~10x past the accuracy gate; sum-preserving quantization
    kills that term at zero device cost (measured: rel 0.0225 -> 0.0014)."""
    a = a.astype(np.float32)
    out = np.empty(a.shape, np.float32)
    carry = np.zeros(a.shape[0], np.float32)
    for d in range(a.shape[1]):
        v = a[:, d] + carry
        q = v.astype(fp8_np).astype(np.float32)
        out[:, d] = q
        carry = v - q
    return out


def _chunk_sizes(Tp: int):
    """Balanced split of Tp token columns into chunks of at most CS.
    Sizes are kept even: odd moving-dims measure ~2% slower per column
    on the PE (alignment penalty). A maximal first chunk (512) was
    tried and measured WORSE (+1.7us): the warmup phase is
    HBM-bandwidth-bound, so a heavier first group delays the pipeline."""
    nch = max(1, math.ceil(Tp / CS))
    base, rem = divmod(Tp, nch)
    sizes = [base + (1 if i < rem else 0) for i in range(nch)]
    for i in range(nch - 1):
        if sizes[i] % 2:
            sizes[i] += 1
            sizes[i + 1] -= 1
    return sizes


def _build_program(Tp: int, mode: str):
    import concourse.tile as tile
    from concourse import bacc, mybir

    sizes = _chunk_sizes(Tp)
    nch = len(sizes)
    offs = [sum(sizes[:i]) for i in range(nch)]  # global token offsets

    f32 = mybir.dt.float32
    bf16 = mybir.dt.bfloat16
    fp8 = mybir.dt.float8e4
    l1_dt = fp8 if mode in ("fp8", "fp8l1") else bf16
    l2_dt = fp8 if mode == "fp8" else bf16
    l1_dr = l1_dt == fp8
    l2_dr = l2_dt == fp8
    dr = mybir.MatmulPerfMode.DoubleRow
    gelu = mybir.ActivationFunctionType.Gelu
    ident = mybir.ActivationFunctionType.Identity

    nc = bacc.Bacc(
        "TRN2", target_bir_lowering=False, debug=False, num_devices=N_CORES
    )

    # xq[c] is the SBUF image of token chunk c: [128, KB_D*CS], row-major
    # (kb, t) per partition, so the DMA is fully contiguous
    xq = nc.dram_tensor(
        "xq", [nch, 128, KB_D * CS], l1_dt, kind="ExternalInput"
    ).ap()
    # boot holds w1[0] k-pair 0 (cols 0:128) + chunk-0 k-pair 0 (cols
    # 128:128+CS): ONE dma + ONE semaphore gates the very first matmul
    boot = nc.dram_tensor(
        "boot", [128, 2, 128 + CS], l1_dt, kind="ExternalInput"
    ).ap()
    # w1[hb] packs TWO h-tiles contiguously per partition (2KB lines):
    # halves the DMA count, descriptor count and completion semaphores
    # on the sync ring vs per-tile loads
    w1 = nc.dram_tensor(
        "w1", [HB // 2, 128, 2 * KB_D * 128], l1_dt, kind="ExternalInput"
    ).ap()
    # w2[db] likewise packs two d-tiles per DMA on the gpsimd ring
    w2 = nc.dram_tensor(
        "w2", [DB // 2, 128, 2 * HB * 128], l2_dt, kind="ExternalInput"
    ).ap()
    b1 = nc.dram_tensor("b1", [128, HB], f32, kind="ExternalInput").ap()
    b2 = nc.dram_tensor("b2", [128, DB], f32, kind="ExternalInput").ap()
    # bf16 output halves the store traffic; the add runs fp32 on DVE and
    # only the final store rounds (costs ~1e-3 rel err, gate is 2e-2)
    yT = nc.dram_tensor("yT", [D, Tp], bf16, kind="ExternalOutput").ap()

    def mm_group(ps, tsz, nk, lhs_of, rhs_of, use_dr):
        """Accumulate nk k-tiles into psum ps[:, :tsz]; DoubleRow fuses
        pairs of k-tiles per matmul via 3D APs."""
        if use_dr:
            for j in range(0, nk, 2):
                nc.tensor.matmul(
                    ps[:, :tsz],
                    lhs_of(j, 2),
                    rhs_of(j, 2),
                    start=(j == 0),
                    stop=(j == nk - 2),
                    perf_mode=dr,
                )
        else:
            for j in range(nk):
                nc.tensor.matmul(
                    ps[:, :tsz],
                    lhs_of(j, 1),
                    rhs_of(j, 1),
                    start=(j == 0),
                    stop=(j == nk - 1),
                )

    with tile.TileContext(nc) as tc:
        with (
            tc.tile_pool(name="const", bufs=1) as const_pool,
            tc.tile_pool(name="acts", bufs=1) as acts_pool,
            tc.tile_pool(name="xtp", bufs=3) as xt_pool,
            tc.tile_pool(name="w1p", bufs=6) as w1_pool,
            tc.tile_pool(name="w2p", bufs=2) as w2_pool,
            tc.tile_pool(name="outp", bufs=4) as out_pool,
            tc.tile_pool(name="psum", bufs=8, space="PSUM") as psum_pool,
        ):
            b1_sb = const_pool.tile([128, HB], f32)
            b2_sb = const_pool.tile([128, DB], f32)

            for sup0 in range(0, nch, SUP // CS):

                cix = list(range(sup0, min(sup0 + SUP // CS, nch)))
                loffs = [offs[c] - offs[cix[0]] for c in cix]  # ht-local
                sup_len = sum(sizes[c] for c in cix)
                ht_sb = acts_pool.tile([128, HB, sup_len], l2_dt, tag="ht")

                # token chunks: chunk 0 on the scalar ring (gates the first
                # matmul; Scalar is idle until GELUs begin), the rest on the
                # gpsimd ring in parallel; the sync ring carries the w1
                # stream so Scalar stays dedicated to GELU afterwards
                # the very first matmul group reads w1[0] k-pair 0 and
                # chunk-0 k-pair 0 out of the boot tile: one DMA, one
                # completion semaphore, first on the gpsimd (SWDGE) ring
                # whose descriptor writes issue earliest and cheapest
                boot_sb = None
                if sup0 == 0 and l1_dr:
                    boot_sb = const_pool.tile([128, 2, 128 + CS], l1_dt)
                    nc.gpsimd.dma_start(boot_sb[:], boot[:])
                    # dependency-free warmup matmuls on (uninitialized)
                    # SBUF: the PE starts immediately and stays busy while
                    # the boot DMA lands, so the first real matmul's wait
                    # is already satisfied when evaluated — hiding the
                    # ~3us DMA-completion semaphore wake latency. Results
                    # land in a psum tile no real group reads; real
                    # groups reset their banks via start=True.
                    garb = const_pool.tile([128, 2, 512], l1_dt)
                    nc.vector.memset(garb[:], 0)
                    warm_ps = psum_pool.tile([128, MM_N], f32, tag="ps")
                    for wi in range(12):
                        nc.tensor.matmul(
                            warm_ps[:, :MM_N],
                            garb[:, :, :128],
                            garb[:, :, :],
                            start=(wi == 0),
                            stop=(wi == 11),
                            perf_mode=dr,
                        )

                xts = []
                for ci, c in enumerate(cix):
                    xt_c = xt_pool.tile(
                        [128, KB_D, CS], l1_dt, tag=f"xt{ci}", bufs=1
                    )
                    src = xq[c].rearrange("p (k m) -> p k m", k=KB_D)
                    if ci == 0:
                        # the first matmul group reads its k-pair 0 from
                        # the boot copy; these normal loads feed h>=1
                        nc.gpsimd.dma_start(xt_c[:, :2], src[:, :2])
                        nc.gpsimd.dma_start(xt_c[:, 2:], src[:, 2:])
                    elif ci == 1:
                        # halves unblock h=0's chunk groups sooner;
                        # chunk 1 rides the scalar ring (idle til GELU)
                        nc.scalar.dma_start(xt_c[:, :4], src[:, :4])
                        nc.scalar.dma_start(xt_c[:, 4:], src[:, 4:])
                    else:
                        nc.gpsimd.dma_start(xt_c[:, :4], src[:, :4])
                        nc.gpsimd.dma_start(xt_c[:, 4:], src[:, 4:])
                    xts.append(xt_c)
                if sup0 == 0:
                    nc.scalar.dma_start(b1_sb[:], b1[:])
                    nc.scalar.dma_start(b2_sb[:], b2[:])

                # ---- layer 1: hT[h, c] ----
                # cell order: the first PA h-tiles run chunk 0 only, with
                # their chunk-1/2 cells deferred — the early phase is
                # HBM-bandwidth-bound, and this gives chunks 1-2 an extra
                # ~3us to land before the PE needs them
                PA = 6
                if len(cix) > 1 and sup0 == 0:
                    cells = (
                        [(h, 0) for h in range(PA)]
                        + [
                            (h, ci)
                            for h in range(PA)
                            for ci in range(1, len(cix))
                        ]
                        + [
                            (h, ci)
                            for h in range(PA, HB)
                            for ci in range(len(cix))
                        ]
                    )
                else:
                    cells = [
                        (h, ci)
                        for h in range(HB)
                        for ci in range(len(cix))
                    ]
                w1blocks = {}
                for h, ci in cells:
                    # w1 stream rides the sync ring so the Scalar queue
                    # stays dedicated to GELU (it was 75% busy with DMA
                    # descriptors + semaphores stealing from ACT)
                    blk = h // 2
                    if blk not in w1blocks:
                        w1t2 = w1_pool.tile(
                            [128, 2, KB_D, 128], l1_dt, tag="w1t"
                        )
                        nc.sync.dma_start(
                            w1t2[:],
                            w1[blk].rearrange(
                                "p (b k m) -> p b k m", b=2, k=KB_D
                            ),
                        )
                        w1blocks[blk] = w1t2
                    w1t = w1blocks[blk][:, h % 2]
                    for ci in [ci]:
                        c = cix[ci]
                        xt_c = xts[ci]
                        tsz = sizes[c]
                        lo = loffs[ci]
                        ps = psum_pool.tile([128, MM_N], f32, tag="ps")
                        if boot_sb is not None and h == 0 and ci == 0:
                            bs = boot_sb

                            def lhs_of(j, w, _w1t=w1t, _bs=bs):
                                if j == 0 and w == 2:
                                    return _bs[:, :, :128]
                                return (
                                    _w1t[:, j : j + w, :]
                                    if w == 2
                                    else _w1t[:, j, :]
                                )

                            def rhs_of(j, w, _xt=xt_c, _bs=bs, _t=tsz):
                                if j == 0 and w == 2:
                                    return _bs[:, :, 128 : 128 + _t]
                                return (
                                    _xt[:, j : j + w, :_t]
                                    if w == 2
                                    else _xt[:, j, :_t]
                                )

                            mm_group(ps, tsz, KB_D, lhs_of, rhs_of, l1_dr)
                        else:
                            mm_group(
                                ps,
                                tsz,
                                KB_D,
                                lambda j, w: w1t[:, j : j + w, :]
                                if w == 2
                                else w1t[:, j, :],
                                lambda j, w: xt_c[:, j : j + w, :tsz]
                                if w == 2
                                else xt_c[:, j, :tsz],
                                l1_dr,
                            )
                        nc.scalar.activation(
                            ht_sb[:, h, lo : lo + tsz],
                            ps[:, :tsz],
                            gelu,
                            bias=b1_sb[:, h : h + 1],
                        )

                # ---- layer 2: yT[d, c] ----
                w2t2 = None
                for d in range(DB):
                    # w2 on the gpsimd (SWDGE) ring: parallel to the w1
                    # stream on the sync ring, so d=0 prefetches early
                    if d % 2 == 0:
                        w2t2 = w2_pool.tile(
                            [128, 2, HB, 128], l2_dt, tag="w2t"
                        )
                        nc.gpsimd.dma_start(
                            w2t2[:],
                            w2[d // 2].rearrange(
                                "p (b k m) -> p b k m", b=2, k=HB
                            ),
                        )
                    w2t = w2t2[:, d % 2]
                    rings = [nc.sync, nc.scalar, nc.gpsimd]
                    # d < DB-1: one DVE add per chunk into a full-width
                    # tile, then ONE store DMA per d-tile (fewer
                    # descriptors + completion semaphores). The final
                    # d-tile keeps fine-grained pieces so the exposed
                    # tail after the last matmul stays short.
                    lo0 = offs[cix[0]]
                    hi0 = offs[cix[-1]] + sizes[cix[-1]]
                    ot_d = None
                    if d < DB - 1:
                        ot_d = out_pool.tile(
                            [128, hi0 - lo0], bf16, tag="otd"
                        )
                    for ci, c in enumerate(cix):
                        tsz = sizes[c]
                        lo = loffs[ci]
                        go = offs[c]
                        # the very last matmul group is split column-wise
                        # so its first half's add+store overlap the second
                        # half's matmuls, shrinking the exposed tail
                        last = d == DB - 1 and c == cix[-1]
                        col_ranges = (
                            [(0, tsz // 2), (tsz // 2, tsz)]
                            if last
                            else [(0, tsz)]
                        )
                        for r0, r1 in col_ranges:
                            rn = r1 - r0
                            ps = psum_pool.tile([128, MM_N], f32, tag="ps")
                            mm_group(
                                ps,
                                rn,
                                HB,
                                lambda j, w: w2t[:, j : j + w, :]
                                if w == 2
                                else w2t[:, j, :],
                                lambda j, w: ht_sb[
                                    :, j : j + w, lo + r0 : lo + r1
                                ]
                                if w == 2
                                else ht_sb[:, j, lo + r0 : lo + r1],
                                l2_dr,
                            )
                            if d < DB - 1:
                                nc.vector.tensor_scalar_add(
                                    ot_d[:, go - lo0 : go - lo0 + tsz],
                                    ps[:, :tsz],
                                    b2_sb[:, d : d + 1],
                                )
                                continue
                            ot = out_pool.tile([128, MM_N], bf16, tag="ot")
                            # final d-tile: split each range's add+store
                            # into pieces on alternating rings so no ring
                            # backlog extends the tail
                            if rn > 160:
                                half = rn // 2
                                pieces = [(0, half), (half, rn - half)]
                            else:
                                pieces = [(0, rn)]
                            for pi, (p0, psz) in enumerate(pieces):
                                # bias add on DVE, not Scalar ACT: keeps
                                # the Scalar queue free and overlaps the
                                # layer-2 tail
                                nc.vector.tensor_scalar_add(
                                    ot[:, p0 : p0 + psz],
                                    ps[:, p0 : p0 + psz],
                                    b2_sb[:, d : d + 1],
                                )
                                st_eng = rings[
                                    (d * len(cix) + ci + pi + r0) % 3
                                ]
                                st_eng.dma_start(
                                    yT[
                                        d * 128 : (d + 1) * 128,
                                        go + r0 + p0 : go + r0 + p0 + psz,
                                    ],
                                    ot[:, p0 : p0 + psz],
                                )
                    if d < DB - 1:
                        rings[d % 3].dma_start(
                            yT[d * 128 : (d + 1) * 128, lo0:hi0],
                            ot_d[:, : hi0 - lo0],
                        )

    nc.compile()
    return nc


def kernel(x, indices_s, weight1, weight2, bias1, bias2):
    from concourse import mybir
    from concourse.bass_utils import run_bass_kernel_spmd

    x = np.asarray(x, dtype=np.float32)
    if MODE in ("fp8", "fp8l1"):
        x = _ed_quant_rows(x, mybir.dt.np(mybir.dt.float8e4))
    idx = np.asarray(indices_s).astype(np.int64).ravel()
    w1_full = np.asarray(weight1, dtype=np.float32)
    w2_full = np.asarray(weight2, dtype=np.float32)
    b1_full = np.asarray(bias1, dtype=np.float32)
    b2_full = np.asarray(bias2, dtype=np.float32)

    order = np.argsort(idx, kind="stable")
    counts = np.bincount(idx, minlength=E)
    starts = np.concatenate([[0], np.cumsum(counts)])
    # tokens live in the free dim everywhere, so no alignment is needed:
    # every core computes exactly max(counts) token columns
    Tp = max(128, int(counts.max()))
    sizes = _chunk_sizes(Tp)
    nch = len(sizes)
    offs = np.concatenate([[0], np.cumsum(sizes)])

    mode = MODE
    key = (Tp, mode)
    nc = _program_cache.get(key)
    if nc is None:
        nc = _build_program(Tp, mode)
        _program_cache[key] = nc

    fp8_np = mybir.dt.np(mybir.dt.float8e4)
    l1_np = fp8_np if mode in ("fp8", "fp8l1") else BF16
    l2_np = fp8_np if mode == "fp8" else BF16

    in_maps = []
    for e in range(E):
        toks = order[starts[e] : starts[e + 1]]
        # slot-aligned image: chunk c's tokens at columns [c*CS, c*CS+sizes[c])
        xTs = np.zeros((D, nch * CS), dtype=np.float32)
        for c in range(nch):
            lo, hi = offs[c], min(offs[c + 1], counts[e])
            if hi > lo:
                xTs[:, c * CS : c * CS + (hi - lo)] = x[toks[lo:hi]].T
        # [D, nch*CS] -> [nch, 128, KB_D*CS] chunk-major SBUF image
        xq = (
            np.ascontiguousarray(
                xTs.reshape(KB_D, 128, nch, CS).transpose(2, 1, 0, 3)
            )
            .reshape(nch, 128, KB_D * CS)
            .astype(l1_np)
        )
        w1r = (
            np.ascontiguousarray(
                w1_full[e].reshape(KB_D, 128, HB, 128).transpose(2, 1, 0, 3)
            )
            .reshape(HB, 128, KB_D * 128)
            .astype(l1_np)
        )
        w2r = (
            np.ascontiguousarray(
                w2_full[e].reshape(HB, 128, DB, 128).transpose(2, 1, 0, 3)
            )
            .reshape(DB, 128, HB * 128)
            .astype(l2_np)
        )
        # pack pairs of tiles contiguously per partition: one DMA (and
        # one completion semaphore) covers two h-/d-tiles
        w1p = np.ascontiguousarray(
            w1r.reshape(HB // 2, 2, 128, KB_D * 128)
            .transpose(0, 2, 1, 3)
            .reshape(HB // 2, 128, 2 * KB_D * 128)
        )
        w2p = np.ascontiguousarray(
            w2r.reshape(DB // 2, 2, 128, HB * 128)
            .transpose(0, 2, 1, 3)
            .reshape(DB // 2, 128, 2 * HB * 128)
        )
        b1d = np.ascontiguousarray(b1_full[e].reshape(HB, 128).T)
        b2d = np.ascontiguousarray(b2_full[e].reshape(DB, 128).T)
        # boot: w1[0] k-pair 0 next to chunk-0 k-pair 0 so one DMA (and
        # one completion semaphore) gates the very first matmul
        bootd = np.concatenate(
            [
                w1r[0].reshape(128, KB_D, 128)[:, :2, :],
                xq[0].reshape(128, KB_D, CS)[:, :2, :],
            ],
            axis=2,
        )
        bootd = np.ascontiguousarray(bootd)
        in_maps.append(
            {
                "xq": xq,
                "w1": w1p,
                "w2": w2p,
                "b1": b1d,
                "b2": b2d,
                "boot": bootd,
            }
        )

    res = run_bass_kernel_spmd(
        nc,
        in_maps,
        list(range(N_CORES)),
        trace=os.environ.get("BASS_TRACE") == "1",
    )
    global last_results
    last_results = res

    out = np.empty((T, D), dtype=np.float32)
    for e in range(E):
        toks = order[starts[e] : starts[e + 1]]
        out[toks] = res.results[e]["yT"][:, : counts[e]].T.astype(np.float32)
    if res.exec_time_ns is not None:
        print(f"HW exec time: {res.exec_time_ns} ns")
    return out[:, None, :]

